# revision 33
# baseline (speedup 1.0000x reference)
"""Trainium2 Bass kernel for nn_LLPKTMultiType (LLPKT knowledge tracing).

Strategy: pure data parallel, 4 samples per core x 8 cores.

The 850-step sequential erase-add memory recurrence
    M_{s+1}[c,d] = M_s[c,d] * (1 - w_s[c] e_s[d]) + w_s[c] a_s[d]
is evaluated with the DVE TensorTensorScan instruction (state = d0*state + d1
along the free axis).  Layout: partitions = (2 samples x 64 d), free =
(50 concepts x step positions), chunked over steps with a zero-multiplier
reset column per concept segment carrying the state across chunks.

v2 perf structure:
  - fp16 datapath for w/E/A/u/v/M (DVE tensor_tensor 2x mode, tensor_scalar
    4x mode, halved broadcast-DMA volume); fp32 accumulators everywhere.
  - gates computed transpose-free: per-sample-half matmuls W_e^T x / W_a^T x
    land PSUM rows directly in the (2b x 64d) partition layout; sigmoid is
    evaluated as (tanh(z/2)+1)/2 so the Act engine only ever needs the
    exp_and_others function table (Exp/Tanh/Copy) - no act-table thrash.
  - w stored to DRAM transposed (c-major) so the per-chunk partition-broadcast
    DMA writes (c,s)-contiguous fp16 tiles (stride-1 inner dim = 2x DVE mode).
  - big per-step readouts (sum_c,k w*M) on GpSimd, small q-readouts on DVE.
"""

import os
import sys

import numpy as np

sys.path.insert(0, "/opt/trn_rl_repo")

B, S, L = 32, 50, 16
C, D = 50, 64
NQ, NL, NU = 10000, 2000, 5000
QV = NQ + NL + 1          # 12001
QAV = 2 * NQ + 1          # 20001
H4 = 4 * D                # 256
EPS = 1e-5

BL = 4                    # samples per core
NCORES = 8
NS = S * (L + 1)          # 850 flat update steps per sample
TC = 8                    # outer timesteps per scan chunk (= gather chunk)
SC = (S + TC - 1) // TC   # 7 chunks (last is ragged: 2 timesteps)
TCNT = [min(TC, S - TC * c) for c in range(SC)]   # 8,8,8,8,8,8,2
CH = 17 * TC              # max update positions per chunk (136)
FW = C * CH               # max w/u/v elements per chunk (c-major)
FM = C * (CH + 1)         # max scan elements incl reset columns
LROWS = S * L             # 800 real lecture rows per sample
LPAD = 896                # padded to 7*128
QPAD = 128

_BUILT = None


def _build():
    import concourse.bass as bass
    import concourse.bacc as bacc
    import concourse.mybir as mybir
    import concourse.tile as tile
    from concourse.masks import make_identity

    f32 = mybir.dt.float32
    f16 = mybir.dt.float16
    i32 = mybir.dt.int32
    AX = mybir.AxisListType
    OP = mybir.AluOpType
    AF = mybir.ActivationFunctionType

    nc = bacc.Bacc("TRN2", target_bir_lowering=False, debug=False,
                   num_devices=NCORES)

    din = lambda n, sh, dt=f32: nc.dram_tensor(n, sh, dt, kind="ExternalInput").ap()
    # gather indices packed column-wise, 10 per sample:
    # b*10 + {0..6: lecture chunks, 7: q, 8: le, 9: qa}
    idx_all = din("idx_all", [128, 40], i32)
    q_embed = din("q_embed", [QV, D])
    qa_embed = din("qa_embed", [QAV, D])
    key = din("key", [C, D])
    M0 = din("M0", [C, D])
    W_ea = din("W_ea", [D, 2 * D])               # W_e | W_a packed
    W0 = din("W0", [H4, H4])
    W1 = din("W1", [H4, H4])
    Wout = din("Wout", [H4])
    biases = din("biases", [2 * H4 + 2 * D])     # b0 | b1 | b_e | b_a
    gbias = din("gbias", [128, 2])               # col0 = b_e/2 (x2 halves), col1 = b_a
    gamma_beta = din("gamma_beta", [2 * H4])
    b_out = din("b_out", [1])                    # already halved on host
    preds = nc.dram_tensor("preds", [BL, S], f32, kind="ExternalOutput").ap()

    with tile.TileContext(nc) as tc:
        with (
            tc.tile_pool(name="persist", bufs=1) as pp,
            tc.tile_pool(name="gather", bufs=3) as gp,
            tc.tile_pool(name="chunk2", bufs=2) as cp2,
            tc.tile_pool(name="chunk1", bufs=1) as cp1,
            tc.tile_pool(name="psum", bufs=2, space="PSUM") as psp,
            tc.tile_pool(name="dram", bufs=1, space="DRAM") as dp,
        ):
            # ---------------- constants / weights ----------------
            ident = pp.tile([128, 128], f32, tag="ident")
            make_identity(nc, ident[:])

            KT = pp.tile([D, C], f32, tag="KT")                    # [d, c]
            nc.sync.dma_start(KT[:], key.rearrange("c d -> d c"))
            Wea_sb = pp.tile([D, 2 * D], f32, tag="Wea")
            nc.sync.dma_start(Wea_sb[:], W_ea)
            gb_sb = pp.tile([128, 2], f32, tag="gb")
            nc.sync.dma_start(gb_sb[:], gbias)
            W01 = pp.tile([128, 4, H4], f32, tag="W01")            # W0lo W0hi W1lo W1hi
            nc.sync.dma_start(W01[:, 0, :], W0[0:128, :])
            nc.sync.dma_start(W01[:, 1, :], W0[128:256, :])
            nc.sync.dma_start(W01[:, 2, :], W1[0:128, :])
            nc.sync.dma_start(W01[:, 3, :], W1[128:256, :])
            Wout_rep = pp.tile([128, H4], f32, tag="Woutr")
            nc.sync.dma_start(Wout_rep[:], Wout[None, :].to_broadcast([128, H4]))
            bias_rep = pp.tile([128, 2 * H4 + 2 * D], f32, tag="biasr")
            nc.sync.dma_start(bias_rep[:],
                              biases[None, :].to_broadcast([128, 2 * H4 + 2 * D]))
            gb_rep = pp.tile([S, 2 * H4], f32, tag="gbr")
            nc.sync.dma_start(gb_rep[:], gamma_beta[None, :].to_broadcast([S, 2 * H4]))
            bout_rep = pp.tile([128, 1], f32, tag="boutr")
            nc.sync.dma_start(bout_rep[:], b_out[None, :].to_broadcast([128, 1]))
            # M0 transposed + replicated: partitions (2b x 64d), free c (fp16)
            M0T = pp.tile([128, C], f16, tag="M0T")
            for bb in range(2):
                nc.gpsimd.dma_start(M0T[D * bb:D * bb + D, :],
                                    M0.rearrange("c d -> d c"))

            # per-sample w: [chunk, c, padded in-chunk position]; each chunk
            # block is contiguous so loads read 13.6KB-contiguous runs
            w_cs = dp.tile([BL, SC, C, CH], f16, tag="wcs")

            qwT = [pp.tile([C, S], f16, tag=f"qwT{b}", name=f"qwT{b}")
                   for b in range(BL)]
            EA = [pp.tile([128, 2, NS], f16, tag=f"EA{p}", name=f"EA{p}")
                  for p in range(2)]
            lr = [pp.tile([128, S], f32, tag=f"lr{p}", name=f"lr{p}") for p in range(2)]
            qr = [pp.tile([128, S], f32, tag=f"qr{p}", name=f"qr{p}") for p in range(2)]

            # ---------------- gather + dense phase ----------------
            idx_sb = pp.tile([128, 40], i32, tag="idxsb")
            nc.sync.dma_start(idx_sb[:], idx_all)

            def gather_cols(cols, table, pool, tagn):
                """out[p, i, :] = table[idx[p, cols[i]]], one indirect DMA per
                column (multi-index indirect DMA miscompiles on HW)."""
                n = cols.stop - cols.start
                g = pool.tile([128, n, D], f32, tag=tagn, name=tagn, bufs=1)
                for i in range(n):
                    nc.gpsimd.indirect_dma_start(
                        out=g[:, i, :], out_offset=None, in_=table,
                        in_offset=bass.IndirectOffsetOnAxis(
                            ap=idx_sb[:, cols.start + i:cols.start + i + 1],
                            axis=0))
                return g

            # one 9-row-group gather per sample from q_embed (lect+q+le),
            # one single-group gather from qa_embed; pair-0 samples first so
            # the first scan chunk's inputs materialize earliest
            g9 = [None] * BL
            gqa = [None] * BL

            def issue_gathers(pr):
                for b in (2 * pr, 2 * pr + 1):
                    g9[b] = gather_cols(slice(10 * b, 10 * b + 9), q_embed,
                                        pp, f"g9_{b}")
                    gqa[b] = gather_cols(slice(10 * b + 9, 10 * b + 10),
                                         qa_embed, gp, f"gqa_{b}")
            issue_gathers(0)

            def xT_of(g):
                ps = psp.tile([D, 128], f32, space="PSUM", tag="tp")
                nc.tensor.transpose(out=ps[:], in_=g, identity=ident[:])
                xT = gp.tile([D, 128], f32, tag="xT")
                nc.scalar.activation(xT[:], ps[:], AF.Copy)
                return xT

            def corr_w(xT, b, dst_view):
                """softmax(x @ K^T) over c -> transpose -> store to w_cs.

                dst_view: target AP in w_cs[b] (c-major positions)."""
                psc = psp.tile([128, C], f32, space="PSUM", tag="corr")
                nc.tensor.matmul(psc[:], lhsT=xT[:], rhs=KT[:],
                                 start=True, stop=True)
                # logits are O(0.1) for this model scale: skip max-subtract
                wexp = gp.tile([128, C], f32, tag="wexp")
                se = gp.tile([128, 1], f32, tag="se")
                nc.scalar.activation(wexp[:], psc[:], AF.Exp,
                                     scale=1.0, accum_out=se[:, :1])
                rse = gp.tile([128, 1], f32, tag="rse")
                nc.vector.reciprocal(rse[:], se[:])
                wsb = gp.tile([128, C], f32, tag="wsb")
                nc.gpsimd.tensor_tensor(wsb[:], wexp[:],
                                        rse[:, 0:1].to_broadcast([128, C]),
                                        op=OP.mult)
                psT = psp.tile([C, 128], f32, space="PSUM", tag="wT", bufs=1)
                nc.tensor.transpose(out=psT[:], in_=wsb[:], identity=ident[:])
                return psT

            def gates_to(psER, psAD, half, xT):
                """matmul W_e^T x and W_a^T x into partition range
                [half, half+64) of the pair-level PSUM accumulators."""
                nc.tensor.matmul(psER[half:half + D, :], lhsT=Wea_sb[:, 0:D],
                                 rhs=xT[:], start=True, stop=True)
                nc.tensor.matmul(psAD[half:half + D, :], lhsT=Wea_sb[:, D:2 * D],
                                 rhs=xT[:], start=True, stop=True)

            def gates_apply(psER, psAD, e_dst, a_dst, nfree):
                """tanh-gates: E holds -sigmoid = -(tanh(z/2)+1)/2; A = tanh(z)."""
                th = gp.tile([128, 128], f16, tag="th")
                nc.scalar.activation(th[:, 0:nfree], psER[:, 0:nfree], AF.Tanh,
                                     bias=gb_sb[:, 0:1], scale=0.5)
                nc.gpsimd.tensor_scalar(e_dst, th[:, 0:nfree], -0.5, -0.5,
                                         op0=OP.mult, op1=OP.add)
                nc.scalar.activation(a_dst, psAD[:, 0:nfree], AF.Tanh,
                                     bias=gb_sb[:, 1:2], scale=1.0)

            # combined u|v tiles (u = first FM elements, v = second), so a
            # single 2x-mode tensor_tensor computes both products per chunk.
            # Reset columns zeroed up-front so nothing blocks the first scan.
            uv_bufs = []
            for i in range(2):
                uv = cp1.tile([128, 2 * FM], f16, tag=f"uv{i}", name=f"uv{i}")
                u3z = uv[:, 0:FM].rearrange("p (c s) -> p c s", s=CH + 1)
                nc.gpsimd.memset(u3z[:, :, 0:1], 0.0)
                uv_bufs.append(uv)
            uv_pitch = [CH + 1, CH + 1]   # current segment pitch per buffer

            def dense(pr, j_range, do_q):
                E3 = EA[pr][:, 0, :].rearrange("p (t k) -> p t k", k=17)
                A3 = EA[pr][:, 1, :].rearrange("p (t k) -> p t k", k=17)
                if not do_q:
                    lecture_chunks(pr, E3, A3, j_range)
                    return

                # question rows first: corr from q_embed, gates from qa_embed
                psER = psp.tile([128, 128], f32, space="PSUM", tag="er", bufs=1)
                psAD = psp.tile([128, 128], f32, space="PSUM", tag="ad", bufs=1)
                for bh in range(2):
                    b = 2 * pr + bh
                    xT = xT_of(g9[b][:, 7, :])
                    psT = corr_w(xT, b, None)
                    nc.scalar.activation(qwT[b][:, 0:S], psT[0:C, 0:S], AF.Copy)

                    xTa = xT_of(gqa[b][:, 0, :])
                    gates_to(psER, psAD, D * bh, xTa)

                gates_apply(psER, psAD, E3[:, 0:S, 16], A3[:, 0:S, 16], S)
                lecture_chunks(pr, E3, A3, j_range)

            def lecture_chunks(pr, E3, A3, j_range):
                # lecture chunks: up to 7 chunks of 128 rows (8t x 16k)/sample
                for j in j_range:
                    t0 = 8 * j
                    tcnt = min(8, S - t0)
                    nfree = tcnt * 16
                    psER = psp.tile([128, 128], f32, space="PSUM", tag="er", bufs=1)
                    psAD = psp.tile([128, 128], f32, space="PSUM", tag="ad", bufs=1)
                    for bh in range(2):
                        b = 2 * pr + bh
                        xT = xT_of(g9[b][:, j, :])
                        gates_to(psER, psAD, D * bh, xT)
                        psT = corr_w(xT, b, None)
                        wf = gp.tile([C, CH], f16, tag="wf")
                        wf3 = wf[:, 0:17 * tcnt].rearrange(
                            "c (t k) -> c t k", k=17)
                        nc.scalar.activation(
                            wf3[:, :, 0:16],
                            psT[0:C, 0:nfree].rearrange(
                                "c (t k) -> c t k", k=16), AF.Copy)
                        nc.scalar.activation(wf3[:, :, 16],
                                             qwT[b][:, t0:t0 + tcnt], AF.Copy)
                        nc.scalar.dma_start(w_cs[b, j, :, 0:17 * tcnt],
                                            wf[:, 0:17 * tcnt])
                    psER3 = psER[:, 0:nfree].rearrange("p (t k) -> p t k", k=16)
                    psAD3 = psAD[:, 0:nfree].rearrange("p (t k) -> p t k", k=16)
                    th = gp.tile([128, 128], f16, tag="th")
                    nc.scalar.activation(
                        th[:, 0:nfree].rearrange("p (t k) -> p t k", k=16),
                        psER3, AF.Tanh, bias=gb_sb[:, 0:1], scale=0.5)
                    nc.gpsimd.tensor_scalar(
                        E3[:, t0:t0 + tcnt, 0:16],
                        th[:, 0:nfree].rearrange("p (t k) -> p t k", k=16),
                        -0.5, -0.5, op0=OP.mult, op1=OP.add)
                    nc.scalar.activation(A3[:, t0:t0 + tcnt, 0:16], psAD3,
                                         AF.Tanh, bias=gb_sb[:, 1:2], scale=1.0)

            # ---------------- scan phase ----------------
            # Per-pair software pipeline: the u/v products + Act complement
            # of chunk ch+2 are emitted before the scan of chunk ch, so the
            # complement hides under scan execution and the DVE queue never
            # waits on the Act queue.
            state = {}           # (pr, ch) -> dict of tiles/views

            def prep(pr, ch):
                tc_ = TCNT[ch]
                chc = 17 * tc_
                s0 = CH * ch
                wb = cp2.tile([128, FW], f16, tag="wb", bufs=3)
                wb3 = wb[:, 0:C * chc].rearrange("p (c s) -> p c s", s=chc)
                for bb in range(2):
                    srcb = w_cs[2 * pr + bb, ch, :, 0:chc]
                    srcb = srcb[None, :, :].to_broadcast([D, C, chc])
                    nc.sync.dma_start(
                        wb[D * bb:D * bb + D, 0:C * chc].rearrange(
                            "p (c s) -> p c s", s=chc), srcb)
                uv = uv_bufs[ch % 2]
                fm = C * (chc + 1)
                u3 = uv[:, 0:fm].rearrange("p (c s) -> p c s", s=chc + 1)
                v3 = uv[:, FM:FM + fm].rearrange("p (c s) -> p c s", s=chc + 1)
                if uv_pitch[ch % 2] != chc + 1:
                    # segment pitch changed since this buffer's last use:
                    # re-zero the u reset columns
                    nc.gpsimd.memset(u3[:, :, 0:1], 0.0)
                    uv_pitch[ch % 2] = chc + 1
                # one fused TT: region 0 = w*E(-er), region 1 = w*A, both
                # broadcast over c; +1 complement for u applied on Act
                uv4 = uv[:].rearrange("p (r f) -> p r f", r=2)
                uv4 = uv4[:, :, :].rearrange("p r (c s) -> p r c s", s=chc + 1) \
                    if False else None
                EA2 = EA[pr][:, :, s0:s0 + chc][:, :, None, :] \
                    .to_broadcast([128, 2, C, chc])
                wb4 = wb[:, 0:C * chc].rearrange("p (c s) -> p c s", s=chc)
                wb4 = wb4[:, None, :, :].to_broadcast([128, 2, C, chc])
                uvo = uv[:].rearrange("p (r f) -> p r f", r=2)[:, :, 0:fm] \
                    .rearrange("p r (c s) -> p r c s", s=chc + 1)
                nc.vector.tensor_tensor(uvo[:, :, :, 1:], wb4, EA2, op=OP.mult)
                nc.vector.tensor_scalar(u3[:, :, 1:], u3[:, :, 1:], 1.0, None,
                                        op0=OP.add)
                state[(pr, ch)] = dict(wb3=wb3, uv=uv, v3=v3, chc=chc, fm=fm)

            def fire(pr, ch):
                st = state[(pr, ch)]
                wb3, uv, v3, chc, fm = (st["wb3"], st["uv"], st["v3"],
                                        st["chc"], st["fm"])
                if ch == 0:
                    nc.vector.tensor_copy(v3[:, :, 0:1], M0T[:][:, :, None])
                else:
                    prev = state[(pr, ch - 1)]
                    nc.vector.tensor_copy(v3[:, :, 0:1], prev["end"])
                Mt = cp2.tile([128, FM], f16, tag="Mt", bufs=2)
                Mt3 = Mt[:, 0:fm].rearrange("p (c s) -> p c s", s=chc + 1)
                nc.vector.tensor_tensor_scan(
                    Mt[:, 0:fm], uv[:, 0:fm], uv[:, FM:FM + fm], 0.0,
                    op0=OP.mult, op1=OP.add)
                st["end"] = Mt3[:, :, chc:chc + 1]
                for tl in range(TCNT[ch]):
                    t = TC * ch + tl
                    sl = 17 * tl
                    scr = cp2.tile([128, C * 16], f16, tag="scr", bufs=3)
                    scr3 = scr[:].rearrange("p (c k) -> p c k", k=16)
                    scr2 = cp2.tile([128, C], f16, tag="scr2")
                    nc.gpsimd.tensor_tensor(scr3, wb3[:, :, sl:sl + 16],
                                            Mt3[:, :, sl:sl + 16], op=OP.mult)
                    nc.scalar.activation(scr[:], scr[:], AF.Copy,
                                         accum_out=lr[pr][:, t:t + 1])
                    nc.vector.scalar_tensor_tensor(
                        out=scr2[:][:, :, None],
                        in0=wb3[:, :, sl + 16:sl + 17],
                        scalar=1.0, op0=OP.mult, in1=Mt3[:, :, sl:sl + 1],
                        op1=OP.mult, accum_out=qr[pr][:, t:t + 1])

            # ---------------- readout: mastery -> LN -> MLP ----------------
            # emitted per pair, right after the pair's scans, so pair-0's
            # readout overlaps pair-1's scan work.  PSUM->SBUF moves go on
            # Act; only the small LN arithmetic runs on DVE.
            msT_lo = pp.tile([128, BL * S], f32, tag="msTlo")
            msT_hi = pp.tile([128, BL * S], f32, tag="msThi")

            def tail(pr):
                ms = pp.tile([S, 2 * H4], f32, tag=f"ms{pr}", name=f"ms{pr}")
                for which, tsrc in ((0, qr[pr]), (2, lr[pr])):
                    pst = psp.tile([S, 128], f32, space="PSUM", tag="tp")
                    nc.tensor.transpose(out=pst[:], in_=tsrc[:], identity=ident[:])
                    for bh in range(2):
                        nc.scalar.activation(
                            ms[:, bh * H4 + which * D:bh * H4 + (which + 1) * D],
                            pst[:, bh * D:(bh + 1) * D], AF.Copy)
                for bh in range(2):
                    b = 2 * pr + bh
                    nc.scalar.activation(ms[:, bh * H4 + D:bh * H4 + 2 * D],
                                         g9[b][0:S, 7, :], AF.Copy)
                    nc.scalar.activation(ms[:, bh * H4 + 3 * D:bh * H4 + 4 * D],
                                         g9[b][0:S, 8, :], AF.Copy)
                ms3 = ms[:].rearrange("p (b f) -> p b f", f=H4)
                mean = pp.tile([S, 2], f32, tag=f"mean{pr}", name=f"mean{pr}")
                nc.vector.tensor_reduce(mean[:], ms3, axis=AX.X, op=OP.add)
                nc.vector.tensor_scalar_mul(mean[:], mean[:], 1.0 / H4)
                mb = mean[:][:, :, None].to_broadcast([S, 2, H4])
                nc.vector.tensor_tensor(ms3, ms3, mb, op=OP.subtract)
                sq = pp.tile([S, 2 * H4], f32, tag=f"sq{pr}", name=f"sq{pr}")
                nc.scalar.activation(sq[:], ms[:], AF.Square)
                var = pp.tile([S, 2], f32, tag=f"var{pr}", name=f"var{pr}")
                nc.vector.tensor_reduce(
                    var[:], sq[:].rearrange("p (b f) -> p b f", f=H4),
                    axis=AX.X, op=OP.add)
                nc.vector.tensor_scalar(var[:], var[:], 1.0 / H4, EPS,
                                        op0=OP.mult, op1=OP.add)
                sd = pp.tile([S, 2], f32, tag=f"sd{pr}", name=f"sd{pr}")
                nc.scalar.activation(sd[:], var[:], AF.Sqrt)
                rsd = pp.tile([S, 2], f32, tag=f"rsd{pr}", name=f"rsd{pr}")
                nc.vector.reciprocal(rsd[:], sd[:])
                nc.vector.tensor_tensor(
                    ms3, ms3, rsd[:][:, :, None].to_broadcast([S, 2, H4]),
                    op=OP.mult)
                gmb = gb_rep[:, 0:H4][:, None, :].to_broadcast([S, 2, H4])
                btb = gb_rep[:, H4:2 * H4][:, None, :].to_broadcast([S, 2, H4])
                nc.vector.tensor_tensor(ms3, ms3, gmb, op=OP.mult)
                nc.vector.tensor_tensor(ms3, ms3, btb, op=OP.add)
                for bh in range(2):
                    b = 2 * pr + bh
                    for fh, dstT in ((0, msT_lo), (1, msT_hi)):
                        pst = psp.tile([128, S], f32, space="PSUM", tag="tp")
                        nc.tensor.transpose(
                            out=pst[:],
                            in_=ms[:, bh * H4 + fh * 128:bh * H4 + (fh + 1) * 128],
                            identity=ident[0:S, 0:S])
                        nc.scalar.activation(dstT[:, b * S:(b + 1) * S], pst[:],
                                             AF.Copy)

                rc = pr
                rows = 2 * S  # 100 rows: (b within pair, t)
                csl = slice(rc * rows, (rc + 1) * rows)
                ph = psp.tile([rows, H4], f32, space="PSUM", tag="mlp", bufs=1)
                nc.tensor.matmul(ph[:], lhsT=msT_lo[:, csl], rhs=W01[:, 0, :],
                                 start=True, stop=False)
                nc.tensor.matmul(ph[:], lhsT=msT_hi[:, csl], rhs=W01[:, 1, :],
                                 start=False, stop=True)
                h1 = pp.tile([rows, H4], f32, tag=f"h1_{rc}", name=f"h1_{rc}")
                nc.vector.tensor_tensor(h1[:], ph[:], bias_rep[0:rows, 0:H4],
                                        op=OP.add)
                nc.scalar.activation(h1[:], h1[:], AF.Relu)
                h1T = [pp.tile([128, rows], f32, tag=f"h1T{fh}_{rc}", name=f"h1T{fh}_{rc}")
                       for fh in range(2)]
                for fh in range(2):
                    pst = psp.tile([128, rows], f32, space="PSUM", tag="tp")
                    nc.tensor.transpose(out=pst[:],
                                        in_=h1[:, fh * 128:(fh + 1) * 128],
                                        identity=ident[0:rows, 0:rows])
                    nc.scalar.activation(h1T[fh][:], pst[:], AF.Copy)
                ph2 = psp.tile([rows, H4], f32, space="PSUM", tag="mlp", bufs=1)
                nc.tensor.matmul(ph2[:], lhsT=h1T[0][:], rhs=W01[:, 2, :],
                                 start=True, stop=False)
                nc.tensor.matmul(ph2[:], lhsT=h1T[1][:], rhs=W01[:, 3, :],
                                 start=False, stop=True)
                h2 = pp.tile([rows, H4], f32, tag=f"h2_{rc}", name=f"h2_{rc}")
                nc.vector.tensor_tensor(h2[:], ph2[:],
                                        bias_rep[0:rows, H4:2 * H4], op=OP.add)
                scr4 = pp.tile([rows, H4], f32, tag=f"scr4_{rc}", name=f"scr4_{rc}")
                logit = pp.tile([rows, 1], f32, tag=f"logit{rc}", name=f"logit{rc}")
                nc.vector.scalar_tensor_tensor(
                    out=scr4[:], in0=h2[:], scalar=1.0, op0=OP.mult,
                    in1=Wout_rep[0:rows, :], op1=OP.mult,
                    accum_out=logit[:, 0:1])
                # sigmoid(z) = (tanh(z/2)+1)/2, with b_out/2 pre-folded in bias
                psig = pp.tile([rows, 1], f32, tag=f"psig{rc}", name=f"psig{rc}")
                nc.scalar.activation(psig[:], logit[:], AF.Tanh,
                                     bias=bout_rep[0:rows, 0:1], scale=0.5)
                nc.vector.tensor_scalar(psig[:], psig[:], 0.5, 0.5,
                                        op0=OP.mult, op1=OP.add)
                nc.sync.dma_start(
                    preds[2 * rc:2 * rc + 2, :].rearrange("b t -> (b t)")[:, None],
                    psig[:, 0:1])


            for pr in range(2):
                if pr == 1:
                    issue_gathers(1)
                dense(pr, range(0, 2), True)    # q + j0 + j1 -> t0..15
                prep(pr, 0)
                prep(pr, 1)
                for ch in range(SC):
                    fire(pr, ch)
                    if ch + 2 < SC:
                        # emit chunk ch+2's dense work just before its prep:
                        # its (tiny) DVE recips slot between scans without
                        # stalling the queue, and stores land just in time
                        dense(pr, range(ch + 2, ch + 3), False)
                        prep(pr, ch + 2)
                    # pair-0's readout/MLP slots in once pair-1's pipeline
                    # is rolling; its PE/DVE pieces then overlap pair-1 scans
                    if pr == 1 and ch == 1:
                        tail(0)
            tail(1)

    nc.compile()
    return nc


def _host_prepare(inputs):
    q_data = np.asarray(inputs["q_data"]).astype(np.int32)
    qa_data = np.asarray(inputs["qa_data"]).astype(np.int32)
    l_data = np.asarray(inputs["l_data"]).astype(np.int32)
    f = lambda k: np.ascontiguousarray(np.asarray(inputs[k]), dtype=np.float32)
    q_embed, qa_embed = f("q_embed"), f("qa_embed")
    key, M0 = f("key_matrix"), f("M0")
    W_ea = np.concatenate([f("W_e"), f("W_a")], axis=1)
    b_e, b_a = f("b_e"), f("b_a")
    biases = np.concatenate([f("b0"), f("b1"), b_e, b_a])
    gbias = np.stack([np.concatenate([b_e / 2, b_e / 2]),
                      np.concatenate([b_a, b_a])], axis=1)
    gamma_beta = np.concatenate([f("ln_gamma"), f("ln_beta")])
    W0, W1 = f("W0"), f("W1")
    Wout = f("W_out").reshape(-1)
    b_out = f("b_out").reshape(-1) / 2.0

    in_maps = []
    for core in range(NCORES):
        bs = slice(core * BL, (core + 1) * BL)
        ql = np.zeros((BL, LPAD), np.int32)
        ql[:, :LROWS] = l_data[bs].reshape(BL, LROWS)
        idx_all = np.zeros((128, 40), np.int32)
        for b in range(BL):
            for j in range(7):
                idx_all[:, b * 10 + j] = ql[b, 128 * j:128 * (j + 1)]
            idx_all[:S, b * 10 + 7] = q_data[bs][b]
            idx_all[:S, b * 10 + 8] = l_data[bs][b, :, L - 1]
            idx_all[:S, b * 10 + 9] = qa_data[bs][b]
        in_maps.append(dict(
            idx_all=np.ascontiguousarray(idx_all),
            q_embed=q_embed, qa_embed=qa_embed, key=key, M0=M0,
            W_ea=W_ea, W0=W0, W1=W1, Wout=Wout, biases=biases,
            gbias=np.ascontiguousarray(gbias),
            gamma_beta=gamma_beta, b_out=b_out,
        ))
    return in_maps


def kernel(**inputs):
    global _BUILT
    if _BUILT is None:
        _BUILT = _build()
    nc = _BUILT
    from concourse import bass_utils
    in_maps = _host_prepare(inputs)
    res = bass_utils.run_bass_kernel_spmd(
        nc, in_maps, core_ids=list(range(NCORES)),
        trace=bool(int(os.environ.get("KERNEL_TRACE", "0"))))
    out = np.concatenate([r["preds"] for r in res.results], axis=0)
    kernel.last_results = res
    return out


# revision 34
# speedup vs baseline: 1.0091x; 1.0091x over previous
"""Trainium2 Bass kernel for nn_LLPKTMultiType (LLPKT knowledge tracing).

Strategy: pure data parallel, 4 samples per core x 8 cores.

The 850-step sequential erase-add memory recurrence
    M_{s+1}[c,d] = M_s[c,d] * (1 - w_s[c] e_s[d]) + w_s[c] a_s[d]
is evaluated with the DVE TensorTensorScan instruction (state = d0*state + d1
along the free axis).  Layout: partitions = (2 samples x 64 d), free =
(50 concepts x step positions), chunked over steps with a zero-multiplier
reset column per concept segment carrying the state across chunks.

v2 perf structure:
  - fp16 datapath for w/E/A/u/v/M (DVE tensor_tensor 2x mode, tensor_scalar
    4x mode, halved broadcast-DMA volume); fp32 accumulators everywhere.
  - gates computed transpose-free: per-sample-half matmuls W_e^T x / W_a^T x
    land PSUM rows directly in the (2b x 64d) partition layout; sigmoid is
    evaluated as (tanh(z/2)+1)/2 so the Act engine only ever needs the
    exp_and_others function table (Exp/Tanh/Copy) - no act-table thrash.
  - w stored to DRAM transposed (c-major) so the per-chunk partition-broadcast
    DMA writes (c,s)-contiguous fp16 tiles (stride-1 inner dim = 2x DVE mode).
  - big per-step readouts (sum_c,k w*M) on GpSimd, small q-readouts on DVE.
"""

import os
import sys

import numpy as np

sys.path.insert(0, "/opt/trn_rl_repo")

B, S, L = 32, 50, 16
C, D = 50, 64
NQ, NL, NU = 10000, 2000, 5000
QV = NQ + NL + 1          # 12001
QAV = 2 * NQ + 1          # 20001
H4 = 4 * D                # 256
EPS = 1e-5

BL = 4                    # samples per core
NCORES = 8
NS = S * (L + 1)          # 850 flat update steps per sample
TC = 8                    # outer timesteps per scan chunk (= gather chunk)
SC = (S + TC - 1) // TC   # 7 chunks (last is ragged: 2 timesteps)
TCNT = [min(TC, S - TC * c) for c in range(SC)]   # 8,8,8,8,8,8,2
CH = 17 * TC              # max update positions per chunk (136)
FW = C * CH               # max w/u/v elements per chunk (c-major)
FM = C * (CH + 1)         # max scan elements incl reset columns
LROWS = S * L             # 800 real lecture rows per sample
LPAD = 896                # padded to 7*128
QPAD = 128

_BUILT = None


def _build():
    import concourse.bass as bass
    import concourse.bacc as bacc
    import concourse.mybir as mybir
    import concourse.tile as tile
    from concourse.masks import make_identity

    f32 = mybir.dt.float32
    f16 = mybir.dt.float16
    i32 = mybir.dt.int32
    AX = mybir.AxisListType
    OP = mybir.AluOpType
    AF = mybir.ActivationFunctionType

    nc = bacc.Bacc("TRN2", target_bir_lowering=False, debug=False,
                   num_devices=NCORES)

    din = lambda n, sh, dt=f32: nc.dram_tensor(n, sh, dt, kind="ExternalInput").ap()
    # gather indices packed column-wise, 10 per sample:
    # b*10 + {0..6: lecture chunks, 7: q, 8: le, 9: qa}
    idx_all = din("idx_all", [128, 40], i32)
    q_embed = din("q_embed", [QV, D])
    qa_embed = din("qa_embed", [QAV, D])
    key = din("key", [C, D])
    M0 = din("M0", [C, D])
    W_ea = din("W_ea", [D, 2 * D])               # W_e | W_a packed
    W0 = din("W0", [H4, H4])
    W1 = din("W1", [H4, H4])
    Wout = din("Wout", [H4])
    biases = din("biases", [2 * H4 + 2 * D])     # b0 | b1 | b_e | b_a
    gbias = din("gbias", [128, 2])               # col0 = b_e/2 (x2 halves), col1 = b_a
    gamma_beta = din("gamma_beta", [2 * H4])
    b_out = din("b_out", [1])                    # already halved on host
    preds = nc.dram_tensor("preds", [BL, S], f32, kind="ExternalOutput").ap()

    with tile.TileContext(nc) as tc:
        with (
            tc.tile_pool(name="persist", bufs=1) as pp,
            tc.tile_pool(name="gather", bufs=3) as gp,
            tc.tile_pool(name="chunk2", bufs=2) as cp2,
            tc.tile_pool(name="chunk1", bufs=1) as cp1,
            tc.tile_pool(name="psum", bufs=2, space="PSUM") as psp,
            tc.tile_pool(name="dram", bufs=1, space="DRAM") as dp,
        ):
            # ---------------- constants / weights ----------------
            ident = pp.tile([128, 128], f32, tag="ident")
            make_identity(nc, ident[:])

            KT = pp.tile([D, C], f32, tag="KT")                    # [d, c]
            nc.sync.dma_start(KT[:], key.rearrange("c d -> d c"))
            Wea_sb = pp.tile([D, 2 * D], f32, tag="Wea")
            nc.sync.dma_start(Wea_sb[:], W_ea)
            gb_sb = pp.tile([128, 2], f32, tag="gb")
            nc.sync.dma_start(gb_sb[:], gbias)
            W01 = pp.tile([128, 4, H4], f32, tag="W01")            # W0lo W0hi W1lo W1hi
            nc.sync.dma_start(W01[:, 0, :], W0[0:128, :])
            nc.sync.dma_start(W01[:, 1, :], W0[128:256, :])
            nc.sync.dma_start(W01[:, 2, :], W1[0:128, :])
            nc.sync.dma_start(W01[:, 3, :], W1[128:256, :])
            Wout_rep = pp.tile([128, H4], f32, tag="Woutr")
            nc.sync.dma_start(Wout_rep[:], Wout[None, :].to_broadcast([128, H4]))
            bias_rep = pp.tile([128, 2 * H4 + 2 * D], f32, tag="biasr")
            nc.sync.dma_start(bias_rep[:],
                              biases[None, :].to_broadcast([128, 2 * H4 + 2 * D]))
            gb_rep = pp.tile([S, 2 * H4], f32, tag="gbr")
            nc.sync.dma_start(gb_rep[:], gamma_beta[None, :].to_broadcast([S, 2 * H4]))
            bout_rep = pp.tile([128, 1], f32, tag="boutr")
            nc.sync.dma_start(bout_rep[:], b_out[None, :].to_broadcast([128, 1]))
            # M0 transposed + replicated: partitions (2b x 64d), free c (fp16)
            M0T = pp.tile([128, C], f16, tag="M0T")
            for bb in range(2):
                nc.gpsimd.dma_start(M0T[D * bb:D * bb + D, :],
                                    M0.rearrange("c d -> d c"))

            # per-sample w: [chunk, c, padded in-chunk position]; each chunk
            # block is contiguous so loads read 13.6KB-contiguous runs
            w_cs = dp.tile([BL, SC, C, CH], f16, tag="wcs")

            qwT = [pp.tile([C, S], f16, tag=f"qwT{b}", name=f"qwT{b}")
                   for b in range(BL)]
            EA = [pp.tile([128, 2, NS], f16, tag=f"EA{p}", name=f"EA{p}")
                  for p in range(2)]
            lr = [pp.tile([128, S], f32, tag=f"lr{p}", name=f"lr{p}") for p in range(2)]
            qr = [pp.tile([128, S], f32, tag=f"qr{p}", name=f"qr{p}") for p in range(2)]

            # ---------------- gather + dense phase ----------------
            idx_sb = pp.tile([128, 40], i32, tag="idxsb")
            nc.sync.dma_start(idx_sb[:], idx_all)

            def gather_cols(cols, table, pool, tagn):
                """out[p, i, :] = table[idx[p, cols[i]]], one indirect DMA per
                column (multi-index indirect DMA miscompiles on HW)."""
                n = cols.stop - cols.start
                g = pool.tile([128, n, D], f32, tag=tagn, name=tagn, bufs=1)
                for i in range(n):
                    nc.gpsimd.indirect_dma_start(
                        out=g[:, i, :], out_offset=None, in_=table,
                        in_offset=bass.IndirectOffsetOnAxis(
                            ap=idx_sb[:, cols.start + i:cols.start + i + 1],
                            axis=0))
                return g

            # per-column gathers, staged so only the critical columns sit
            # ahead of the first scans in the Pool queue
            g9 = [pp.tile([128, 9, D], f32, tag=f"g9_{b}", name=f"g9_{b}")
                  for b in range(BL)]
            gqa = [pp.tile([128, 1, D], f32, tag=f"gqa_{b}", name=f"gqa_{b}")
                   for b in range(BL)]

            def issue_gather(b, i):
                dst = gqa[b][:, 0, :] if i == 9 else g9[b][:, i, :]
                table = qa_embed if i == 9 else q_embed
                nc.gpsimd.indirect_dma_start(
                    out=dst, out_offset=None, in_=table,
                    in_offset=bass.IndirectOffsetOnAxis(
                        ap=idx_sb[:, 10 * b + i:10 * b + i + 1], axis=0))

            def xT_of(g):
                ps = psp.tile([D, 128], f32, space="PSUM", tag="tp")
                nc.tensor.transpose(out=ps[:], in_=g, identity=ident[:])
                xT = gp.tile([D, 128], f32, tag="xT")
                nc.scalar.activation(xT[:], ps[:], AF.Copy)
                return xT

            def corr_w(xT, b, dst_view):
                """softmax(x @ K^T) over c -> transpose -> store to w_cs.

                dst_view: target AP in w_cs[b] (c-major positions)."""
                psc = psp.tile([128, C], f32, space="PSUM", tag="corr")
                nc.tensor.matmul(psc[:], lhsT=xT[:], rhs=KT[:],
                                 start=True, stop=True)
                # logits are O(0.1) for this model scale: skip max-subtract
                wexp = gp.tile([128, C], f32, tag="wexp")
                se = gp.tile([128, 1], f32, tag="se")
                nc.scalar.activation(wexp[:], psc[:], AF.Exp,
                                     scale=1.0, accum_out=se[:, :1])
                rse = gp.tile([128, 1], f32, tag="rse")
                nc.vector.reciprocal(rse[:], se[:])
                wsb = gp.tile([128, C], f32, tag="wsb")
                nc.gpsimd.tensor_tensor(wsb[:], wexp[:],
                                        rse[:, 0:1].to_broadcast([128, C]),
                                        op=OP.mult)
                psT = psp.tile([C, 128], f32, space="PSUM", tag="wT", bufs=1)
                nc.tensor.transpose(out=psT[:], in_=wsb[:], identity=ident[:])
                return psT

            def gates_to(psER, psAD, half, xT):
                """matmul W_e^T x and W_a^T x into partition range
                [half, half+64) of the pair-level PSUM accumulators."""
                nc.tensor.matmul(psER[half:half + D, :], lhsT=Wea_sb[:, 0:D],
                                 rhs=xT[:], start=True, stop=True)
                nc.tensor.matmul(psAD[half:half + D, :], lhsT=Wea_sb[:, D:2 * D],
                                 rhs=xT[:], start=True, stop=True)

            def gates_apply(psER, psAD, e_dst, a_dst, nfree):
                """tanh-gates: E holds -sigmoid = -(tanh(z/2)+1)/2; A = tanh(z)."""
                th = gp.tile([128, 128], f16, tag="th")
                nc.scalar.activation(th[:, 0:nfree], psER[:, 0:nfree], AF.Tanh,
                                     bias=gb_sb[:, 0:1], scale=0.5)
                nc.gpsimd.tensor_scalar(e_dst, th[:, 0:nfree], -0.5, -0.5,
                                         op0=OP.mult, op1=OP.add)
                nc.scalar.activation(a_dst, psAD[:, 0:nfree], AF.Tanh,
                                     bias=gb_sb[:, 1:2], scale=1.0)

            # combined u|v tiles (u = first FM elements, v = second), so a
            # single 2x-mode tensor_tensor computes both products per chunk.
            # Reset columns zeroed up-front so nothing blocks the first scan.
            uv_bufs = []
            for i in range(2):
                uv = cp1.tile([128, 2 * FM], f16, tag=f"uv{i}", name=f"uv{i}")
                u3z = uv[:, 0:FM].rearrange("p (c s) -> p c s", s=CH + 1)
                nc.gpsimd.memset(u3z[:, :, 0:1], 0.0)
                uv_bufs.append(uv)
            uv_pitch = [CH + 1, CH + 1]   # current segment pitch per buffer

            def dense(pr, j_range, do_q):
                E3 = EA[pr][:, 0, :].rearrange("p (t k) -> p t k", k=17)
                A3 = EA[pr][:, 1, :].rearrange("p (t k) -> p t k", k=17)
                if not do_q:
                    lecture_chunks(pr, E3, A3, j_range)
                    return

                # question rows first: corr from q_embed, gates from qa_embed
                psER = psp.tile([128, 128], f32, space="PSUM", tag="er", bufs=1)
                psAD = psp.tile([128, 128], f32, space="PSUM", tag="ad", bufs=1)
                for bh in range(2):
                    b = 2 * pr + bh
                    xT = xT_of(g9[b][:, 7, :])
                    psT = corr_w(xT, b, None)
                    nc.scalar.activation(qwT[b][:, 0:S], psT[0:C, 0:S], AF.Copy)

                    xTa = xT_of(gqa[b][:, 0, :])
                    gates_to(psER, psAD, D * bh, xTa)

                gates_apply(psER, psAD, E3[:, 0:S, 16], A3[:, 0:S, 16], S)
                lecture_chunks(pr, E3, A3, j_range)

            def lecture_chunks(pr, E3, A3, j_range):
                # lecture chunks: up to 7 chunks of 128 rows (8t x 16k)/sample
                for j in j_range:
                    t0 = 8 * j
                    tcnt = min(8, S - t0)
                    nfree = tcnt * 16
                    psER = psp.tile([128, 128], f32, space="PSUM", tag="er", bufs=1)
                    psAD = psp.tile([128, 128], f32, space="PSUM", tag="ad", bufs=1)
                    for bh in range(2):
                        b = 2 * pr + bh
                        xT = xT_of(g9[b][:, j, :])
                        gates_to(psER, psAD, D * bh, xT)
                        psT = corr_w(xT, b, None)
                        wf = gp.tile([C, CH], f16, tag="wf")
                        wf3 = wf[:, 0:17 * tcnt].rearrange(
                            "c (t k) -> c t k", k=17)
                        nc.scalar.activation(
                            wf3[:, :, 0:16],
                            psT[0:C, 0:nfree].rearrange(
                                "c (t k) -> c t k", k=16), AF.Copy)
                        nc.scalar.activation(wf3[:, :, 16],
                                             qwT[b][:, t0:t0 + tcnt], AF.Copy)
                        nc.scalar.dma_start(w_cs[b, j, :, 0:17 * tcnt],
                                            wf[:, 0:17 * tcnt])
                    psER3 = psER[:, 0:nfree].rearrange("p (t k) -> p t k", k=16)
                    psAD3 = psAD[:, 0:nfree].rearrange("p (t k) -> p t k", k=16)
                    th = gp.tile([128, 128], f16, tag="th")
                    nc.scalar.activation(
                        th[:, 0:nfree].rearrange("p (t k) -> p t k", k=16),
                        psER3, AF.Tanh, bias=gb_sb[:, 0:1], scale=0.5)
                    nc.gpsimd.tensor_scalar(
                        E3[:, t0:t0 + tcnt, 0:16],
                        th[:, 0:nfree].rearrange("p (t k) -> p t k", k=16),
                        -0.5, -0.5, op0=OP.mult, op1=OP.add)
                    nc.scalar.activation(A3[:, t0:t0 + tcnt, 0:16], psAD3,
                                         AF.Tanh, bias=gb_sb[:, 1:2], scale=1.0)

            # ---------------- scan phase ----------------
            # Per-pair software pipeline: the u/v products + Act complement
            # of chunk ch+2 are emitted before the scan of chunk ch, so the
            # complement hides under scan execution and the DVE queue never
            # waits on the Act queue.
            state = {}           # (pr, ch) -> dict of tiles/views

            def prep(pr, ch):
                tc_ = TCNT[ch]
                chc = 17 * tc_
                s0 = CH * ch
                wb = cp2.tile([128, FW], f16, tag="wb", bufs=3)
                wb3 = wb[:, 0:C * chc].rearrange("p (c s) -> p c s", s=chc)
                for bb in range(2):
                    srcb = w_cs[2 * pr + bb, ch, :, 0:chc]
                    srcb = srcb[None, :, :].to_broadcast([D, C, chc])
                    nc.sync.dma_start(
                        wb[D * bb:D * bb + D, 0:C * chc].rearrange(
                            "p (c s) -> p c s", s=chc), srcb)
                uv = uv_bufs[ch % 2]
                fm = C * (chc + 1)
                u3 = uv[:, 0:fm].rearrange("p (c s) -> p c s", s=chc + 1)
                v3 = uv[:, FM:FM + fm].rearrange("p (c s) -> p c s", s=chc + 1)
                if uv_pitch[ch % 2] != chc + 1:
                    # segment pitch changed since this buffer's last use:
                    # re-zero the u reset columns
                    nc.gpsimd.memset(u3[:, :, 0:1], 0.0)
                    uv_pitch[ch % 2] = chc + 1
                # one fused TT: region 0 = w*E(-er), region 1 = w*A, both
                # broadcast over c; +1 complement for u applied on Act
                uv4 = uv[:].rearrange("p (r f) -> p r f", r=2)
                uv4 = uv4[:, :, :].rearrange("p r (c s) -> p r c s", s=chc + 1) \
                    if False else None
                EA2 = EA[pr][:, :, s0:s0 + chc][:, :, None, :] \
                    .to_broadcast([128, 2, C, chc])
                wb4 = wb[:, 0:C * chc].rearrange("p (c s) -> p c s", s=chc)
                wb4 = wb4[:, None, :, :].to_broadcast([128, 2, C, chc])
                uvo = uv[:].rearrange("p (r f) -> p r f", r=2)[:, :, 0:fm] \
                    .rearrange("p r (c s) -> p r c s", s=chc + 1)
                nc.vector.tensor_tensor(uvo[:, :, :, 1:], wb4, EA2, op=OP.mult)
                nc.vector.tensor_scalar(u3[:, :, 1:], u3[:, :, 1:], 1.0, None,
                                        op0=OP.add)
                state[(pr, ch)] = dict(wb3=wb3, uv=uv, v3=v3, chc=chc, fm=fm)

            def fire(pr, ch):
                st = state[(pr, ch)]
                wb3, uv, v3, chc, fm = (st["wb3"], st["uv"], st["v3"],
                                        st["chc"], st["fm"])
                if ch == 0:
                    nc.vector.tensor_copy(v3[:, :, 0:1], M0T[:][:, :, None])
                else:
                    prev = state[(pr, ch - 1)]
                    nc.vector.tensor_copy(v3[:, :, 0:1], prev["end"])
                Mt = cp2.tile([128, FM], f16, tag="Mt", bufs=2)
                Mt3 = Mt[:, 0:fm].rearrange("p (c s) -> p c s", s=chc + 1)
                nc.vector.tensor_tensor_scan(
                    Mt[:, 0:fm], uv[:, 0:fm], uv[:, FM:FM + fm], 0.0,
                    op0=OP.mult, op1=OP.add)
                st["end"] = Mt3[:, :, chc:chc + 1]
                for tl in range(TCNT[ch]):
                    t = TC * ch + tl
                    sl = 17 * tl
                    scr = cp2.tile([128, C * 16], f16, tag="scr", bufs=3)
                    scr3 = scr[:].rearrange("p (c k) -> p c k", k=16)
                    scr2 = cp2.tile([128, C], f16, tag="scr2")
                    nc.gpsimd.tensor_tensor(scr3, wb3[:, :, sl:sl + 16],
                                            Mt3[:, :, sl:sl + 16], op=OP.mult)
                    nc.scalar.activation(scr[:], scr[:], AF.Copy,
                                         accum_out=lr[pr][:, t:t + 1])
                    nc.vector.scalar_tensor_tensor(
                        out=scr2[:][:, :, None],
                        in0=wb3[:, :, sl + 16:sl + 17],
                        scalar=1.0, op0=OP.mult, in1=Mt3[:, :, sl:sl + 1],
                        op1=OP.mult, accum_out=qr[pr][:, t:t + 1])

            # ---------------- readout: mastery -> LN -> MLP ----------------
            # emitted per pair, right after the pair's scans, so pair-0's
            # readout overlaps pair-1's scan work.  PSUM->SBUF moves go on
            # Act; only the small LN arithmetic runs on DVE.
            msT_lo = pp.tile([128, BL * S], f32, tag="msTlo")
            msT_hi = pp.tile([128, BL * S], f32, tag="msThi")

            def tail(pr):
                ms = pp.tile([S, 2 * H4], f32, tag=f"ms{pr}", name=f"ms{pr}")
                for which, tsrc in ((0, qr[pr]), (2, lr[pr])):
                    pst = psp.tile([S, 128], f32, space="PSUM", tag="tp")
                    nc.tensor.transpose(out=pst[:], in_=tsrc[:], identity=ident[:])
                    for bh in range(2):
                        nc.scalar.activation(
                            ms[:, bh * H4 + which * D:bh * H4 + (which + 1) * D],
                            pst[:, bh * D:(bh + 1) * D], AF.Copy)
                for bh in range(2):
                    b = 2 * pr + bh
                    nc.scalar.activation(ms[:, bh * H4 + D:bh * H4 + 2 * D],
                                         g9[b][0:S, 7, :], AF.Copy)
                    nc.scalar.activation(ms[:, bh * H4 + 3 * D:bh * H4 + 4 * D],
                                         g9[b][0:S, 8, :], AF.Copy)
                ms3 = ms[:].rearrange("p (b f) -> p b f", f=H4)
                mean = pp.tile([S, 2], f32, tag=f"mean{pr}", name=f"mean{pr}")
                nc.vector.tensor_reduce(mean[:], ms3, axis=AX.X, op=OP.add)
                nc.vector.tensor_scalar_mul(mean[:], mean[:], 1.0 / H4)
                mb = mean[:][:, :, None].to_broadcast([S, 2, H4])
                nc.vector.tensor_tensor(ms3, ms3, mb, op=OP.subtract)
                sq = pp.tile([S, 2 * H4], f32, tag=f"sq{pr}", name=f"sq{pr}")
                nc.scalar.activation(sq[:], ms[:], AF.Square)
                var = pp.tile([S, 2], f32, tag=f"var{pr}", name=f"var{pr}")
                nc.vector.tensor_reduce(
                    var[:], sq[:].rearrange("p (b f) -> p b f", f=H4),
                    axis=AX.X, op=OP.add)
                nc.vector.tensor_scalar(var[:], var[:], 1.0 / H4, EPS,
                                        op0=OP.mult, op1=OP.add)
                sd = pp.tile([S, 2], f32, tag=f"sd{pr}", name=f"sd{pr}")
                nc.scalar.activation(sd[:], var[:], AF.Sqrt)
                rsd = pp.tile([S, 2], f32, tag=f"rsd{pr}", name=f"rsd{pr}")
                nc.vector.reciprocal(rsd[:], sd[:])
                nc.vector.tensor_tensor(
                    ms3, ms3, rsd[:][:, :, None].to_broadcast([S, 2, H4]),
                    op=OP.mult)
                gmb = gb_rep[:, 0:H4][:, None, :].to_broadcast([S, 2, H4])
                btb = gb_rep[:, H4:2 * H4][:, None, :].to_broadcast([S, 2, H4])
                nc.vector.tensor_tensor(ms3, ms3, gmb, op=OP.mult)
                nc.vector.tensor_tensor(ms3, ms3, btb, op=OP.add)
                for bh in range(2):
                    b = 2 * pr + bh
                    for fh, dstT in ((0, msT_lo), (1, msT_hi)):
                        pst = psp.tile([128, S], f32, space="PSUM", tag="tp")
                        nc.tensor.transpose(
                            out=pst[:],
                            in_=ms[:, bh * H4 + fh * 128:bh * H4 + (fh + 1) * 128],
                            identity=ident[0:S, 0:S])
                        nc.scalar.activation(dstT[:, b * S:(b + 1) * S], pst[:],
                                             AF.Copy)

                rc = pr
                rows = 2 * S  # 100 rows: (b within pair, t)
                csl = slice(rc * rows, (rc + 1) * rows)
                ph = psp.tile([rows, H4], f32, space="PSUM", tag="mlp", bufs=1)
                nc.tensor.matmul(ph[:], lhsT=msT_lo[:, csl], rhs=W01[:, 0, :],
                                 start=True, stop=False)
                nc.tensor.matmul(ph[:], lhsT=msT_hi[:, csl], rhs=W01[:, 1, :],
                                 start=False, stop=True)
                h1 = pp.tile([rows, H4], f32, tag=f"h1_{rc}", name=f"h1_{rc}")
                nc.vector.tensor_tensor(h1[:], ph[:], bias_rep[0:rows, 0:H4],
                                        op=OP.add)
                nc.scalar.activation(h1[:], h1[:], AF.Relu)
                h1T = [pp.tile([128, rows], f32, tag=f"h1T{fh}_{rc}", name=f"h1T{fh}_{rc}")
                       for fh in range(2)]
                for fh in range(2):
                    pst = psp.tile([128, rows], f32, space="PSUM", tag="tp")
                    nc.tensor.transpose(out=pst[:],
                                        in_=h1[:, fh * 128:(fh + 1) * 128],
                                        identity=ident[0:rows, 0:rows])
                    nc.scalar.activation(h1T[fh][:], pst[:], AF.Copy)
                ph2 = psp.tile([rows, H4], f32, space="PSUM", tag="mlp", bufs=1)
                nc.tensor.matmul(ph2[:], lhsT=h1T[0][:], rhs=W01[:, 2, :],
                                 start=True, stop=False)
                nc.tensor.matmul(ph2[:], lhsT=h1T[1][:], rhs=W01[:, 3, :],
                                 start=False, stop=True)
                h2 = pp.tile([rows, H4], f32, tag=f"h2_{rc}", name=f"h2_{rc}")
                nc.vector.tensor_tensor(h2[:], ph2[:],
                                        bias_rep[0:rows, H4:2 * H4], op=OP.add)
                scr4 = pp.tile([rows, H4], f32, tag=f"scr4_{rc}", name=f"scr4_{rc}")
                logit = pp.tile([rows, 1], f32, tag=f"logit{rc}", name=f"logit{rc}")
                nc.vector.scalar_tensor_tensor(
                    out=scr4[:], in0=h2[:], scalar=1.0, op0=OP.mult,
                    in1=Wout_rep[0:rows, :], op1=OP.mult,
                    accum_out=logit[:, 0:1])
                # sigmoid(z) = (tanh(z/2)+1)/2, with b_out/2 pre-folded in bias
                psig = pp.tile([rows, 1], f32, tag=f"psig{rc}", name=f"psig{rc}")
                nc.scalar.activation(psig[:], logit[:], AF.Tanh,
                                     bias=bout_rep[0:rows, 0:1], scale=0.5)
                nc.vector.tensor_scalar(psig[:], psig[:], 0.5, 0.5,
                                        op0=OP.mult, op1=OP.add)
                nc.sync.dma_start(
                    preds[2 * rc:2 * rc + 2, :].rearrange("b t -> (b t)")[:, None],
                    psig[:, 0:1])


            for pr in range(2):
                # critical gather columns first: q, qa, j0, j1
                for b in (2 * pr, 2 * pr + 1):
                    for i in (7, 9, 0, 1):
                        issue_gather(b, i)
                dense(pr, range(0, 2), True)    # q + j0 + j1 -> t0..15
                prep(pr, 0)
                prep(pr, 1)
                for b in (2 * pr, 2 * pr + 1):
                    issue_gather(b, 2)
                    issue_gather(b, 3)
                for ch in range(SC):
                    fire(pr, ch)
                    if ch < 3:
                        for b in (2 * pr, 2 * pr + 1):
                            issue_gather(b, 4 + ch)     # cols 4,5,6
                            if ch == 2:
                                issue_gather(b, 8)      # le rows
                    if ch + 2 < SC:
                        # emit chunk ch+2's dense work just before its prep:
                        # its (tiny) DVE recips slot between scans without
                        # stalling the queue, and stores land just in time
                        dense(pr, range(ch + 2, ch + 3), False)
                        prep(pr, ch + 2)
                    # pair-0's readout/MLP slots in once pair-1's pipeline
                    # is rolling; its PE/DVE pieces then overlap pair-1 scans
                    if pr == 1 and ch == 1:
                        tail(0)
            tail(1)

    nc.compile()
    return nc


def _host_prepare(inputs):
    q_data = np.asarray(inputs["q_data"]).astype(np.int32)
    qa_data = np.asarray(inputs["qa_data"]).astype(np.int32)
    l_data = np.asarray(inputs["l_data"]).astype(np.int32)
    f = lambda k: np.ascontiguousarray(np.asarray(inputs[k]), dtype=np.float32)
    q_embed, qa_embed = f("q_embed"), f("qa_embed")
    key, M0 = f("key_matrix"), f("M0")
    W_ea = np.concatenate([f("W_e"), f("W_a")], axis=1)
    b_e, b_a = f("b_e"), f("b_a")
    biases = np.concatenate([f("b0"), f("b1"), b_e, b_a])
    gbias = np.stack([np.concatenate([b_e / 2, b_e / 2]),
                      np.concatenate([b_a, b_a])], axis=1)
    gamma_beta = np.concatenate([f("ln_gamma"), f("ln_beta")])
    W0, W1 = f("W0"), f("W1")
    Wout = f("W_out").reshape(-1)
    b_out = f("b_out").reshape(-1) / 2.0

    in_maps = []
    for core in range(NCORES):
        bs = slice(core * BL, (core + 1) * BL)
        ql = np.zeros((BL, LPAD), np.int32)
        ql[:, :LROWS] = l_data[bs].reshape(BL, LROWS)
        idx_all = np.zeros((128, 40), np.int32)
        for b in range(BL):
            for j in range(7):
                idx_all[:, b * 10 + j] = ql[b, 128 * j:128 * (j + 1)]
            idx_all[:S, b * 10 + 7] = q_data[bs][b]
            idx_all[:S, b * 10 + 8] = l_data[bs][b, :, L - 1]
            idx_all[:S, b * 10 + 9] = qa_data[bs][b]
        in_maps.append(dict(
            idx_all=np.ascontiguousarray(idx_all),
            q_embed=q_embed, qa_embed=qa_embed, key=key, M0=M0,
            W_ea=W_ea, W0=W0, W1=W1, Wout=Wout, biases=biases,
            gbias=np.ascontiguousarray(gbias),
            gamma_beta=gamma_beta, b_out=b_out,
        ))
    return in_maps


def kernel(**inputs):
    global _BUILT
    if _BUILT is None:
        _BUILT = _build()
    nc = _BUILT
    from concourse import bass_utils
    in_maps = _host_prepare(inputs)
    res = bass_utils.run_bass_kernel_spmd(
        nc, in_maps, core_ids=list(range(NCORES)),
        trace=bool(int(os.environ.get("KERNEL_TRACE", "0"))))
    out = np.concatenate([r["preds"] for r in res.results], axis=0)
    kernel.last_results = res
    return out
